# revision 28
# baseline (speedup 1.0000x reference)
"""Causal attention (B=8, S=2048, D=768, single head) on 8 trn2 NeuronCores.

Sharding: data-parallel over batch — core b computes batch element b.
Per-core pipeline (all fused over 512-wide s-chunks to bound SBUF):
  1. transpose x chunk via PE+identity -> xT [d_part, s]
  2. projections: KT/QT as [e_part, s] (lhsT = w col-block, rhs = xT),
     V natural [s_part, e] (lhsT = xT block, rhs = wv) + ones column
  3. scores^T tile [k_part, q] = KT_blk.T @ QT_chunk (contract e)
  4. exp (scale fused) on ACT, causal mask on partial diagonal blocks
  5. out[q,e] (+denominator via ones col) = PT_blk.T @ V_blk (contract k)
  6. normalize by reciprocal of denominator column, DMA out

Matmul operands are stored as float32r (full-rate PE, producers round on
write) by default; KMM=fp32 switches to exact fp32 (4x slower matmuls).
"""

import os

import numpy as np

import concourse.bass as bass
import concourse.mybir as mybir
from concourse import bacc
from concourse.tile import TileContext
from concourse.bass_utils import run_bass_kernel_spmd
from concourse.masks import make_identity

B, S, D = 8, 2048, 768
P = 128
ND = D // P            # 6 feature blocks
NB = S // P            # 16 seq blocks
CH = 512               # s-chunk width
NCH = S // CH          # 4 chunks
QPC = CH // P          # 4 q-blocks per chunk
SCALE = 1.0 / float(np.sqrt(D))
F32 = mybir.dt.float32

MM_MODE = os.environ.get("KMM", "f32r")  # f32r | fp32
MDT = mybir.dt.float32r if MM_MODE == "f32r" else F32


def _build_nc():
    nc = bacc.Bacc(None, target_bir_lowering=False)
    xb = nc.dram_tensor("xb", [S, D], F32, kind="ExternalInput")
    wq_d = nc.dram_tensor("wq", [D, D], F32, kind="ExternalInput")
    wk_d = nc.dram_tensor("wk", [D, D], F32, kind="ExternalInput")
    wv_d = nc.dram_tensor("wv", [D, D], F32, kind="ExternalInput")
    out_d = nc.dram_tensor("out", [S, D], F32, kind="ExternalOutput")

    wq_r = wq_d[:, :].rearrange("(o p) e -> p o e", p=P).bitcast(MDT)
    wk_r = wk_d[:, :].rearrange("(o p) e -> p o e", p=P).bitcast(MDT)
    wv_r = wv_d[:, :].rearrange("(o p) e -> p o e", p=P).bitcast(MDT)

    with TileContext(nc) as tc:
        with (
            tc.tile_pool(name="const", bufs=1) as constp,
            tc.tile_pool(name="persist", bufs=1) as persist,
            tc.tile_pool(name="xload", bufs=1) as xload,
            tc.tile_pool(name="xt", bufs=1) as xtp,
            tc.tile_pool(name="qt", bufs=1) as qtp,
            tc.tile_pool(name="wload", bufs=2) as wload,
            tc.tile_pool(name="pt", bufs=1) as ptp,
            tc.tile_pool(name="outp", bufs=2) as outp,
            tc.tile_pool(name="rc", bufs=4) as rcp,
            tc.tile_pool(name="psA", bufs=2, space="PSUM") as psA,
            tc.tile_pool(name="psQKV", bufs=3, space="PSUM") as psQKV,
            tc.tile_pool(name="psO", bufs=3, space="PSUM") as psO,
        ):
            ident = constp.tile([P, P], F32)
            make_identity(nc, ident)
            ident_r = constp.tile([P, P], MDT)
            nc.vector.tensor_copy(ident_r, ident)
            # smask[p, g] = 1.0 if p <= g - 128 else 0.0; slices give the
            # partial-diagonal causal masks for S^T tiles.
            smask = constp.tile([P, 640], F32)
            nc.gpsimd.memset(smask, 1.0)
            nc.gpsimd.affine_select(
                out=smask,
                in_=smask,
                compare_op=mybir.AluOpType.is_ge,
                fill=0.0,
                base=-128,
                pattern=[[1, 640]],
                channel_multiplier=-1,
            )

            KT = persist.tile([P, ND, S], MDT)       # K^T: [e_in, eo, s]
            V = persist.tile([P, NB, D + 2], MDT)    # [s_in, sb, e]; cols D,D+1 = 1.0
            WV = persist.tile([P, ND, D], MDT)       # wv: [d_in, do, e]
            ones_col = constp.tile([P, NB, 2], F32)
            nc.vector.memset(ones_col, 1.0)
            nc.vector.tensor_copy(V[:, :, D : D + 2], ones_col)

            pt_tiles = {}

            def load_x(cc, split_first=False):
                tiles = []
                for sb4 in range(QPC):
                    xt_ld = xload.tile([P, D], MDT, tag=f"x{sb4}")
                    s0 = (cc * QPC + sb4) * P
                    if sb4 == 0 and split_first:
                        nc.sync.dma_start(xt_ld[:, 0 : D // 2], xb[s0 : s0 + P, 0 : D // 2].bitcast(MDT))
                        nc.sync.dma_start(xt_ld[:, D // 2 : D], xb[s0 : s0 + P, D // 2 : D].bitcast(MDT))
                    else:
                        nc.sync.dma_start(xt_ld[:], xb[s0 : s0 + P, :].bitcast(MDT))
                    tiles.append(xt_ld)
                return tiles

            xtiles = load_x(0, split_first=True)
            for c in range(NCH):
                # ---- transpose this s-chunk into xTc [d, s_local]
                xTc = xtp.tile([P, ND, CH], MDT, tag="xT")
                for sb4 in range(QPC):
                    for h in range(2):
                        ps_t = psQKV.tile([P, CH], MDT, tag="qkv")
                        for dh in range(3):
                            do = h * 3 + dh
                            nc.tensor.transpose(
                                ps_t[:, dh * P : (dh + 1) * P],
                                xtiles[sb4][:, do * P : (do + 1) * P],
                                ident_r,
                            )
                        nc.vector.tensor_copy(
                            xTc[:, h * 3 : h * 3 + 3, sb4 * P : (sb4 + 1) * P],
                            ps_t[:, 0 : 3 * P].rearrange("p (o s) -> p o s", o=3),
                        )

                # ---- projections for this chunk
                QTc = qtp.tile([P, ND, CH], MDT, tag="qt")
                for eb in range(ND):
                    wq_t = wload.tile([P, ND, P], MDT, tag="wq")
                    nc.scalar.dma_start(wq_t[:], wq_r[:, :, eb * P : (eb + 1) * P])
                    wk_t = wload.tile([P, ND, P], MDT, tag="wk")
                    nc.sync.dma_start(wk_t[:], wk_r[:, :, eb * P : (eb + 1) * P])
                    if eb == 1 and c + 1 < NCH:
                        xtiles = load_x(c + 1)
                    if eb == 3 and c == 0:
                        nc.gpsimd.dma_start(WV[:], wv_r)
                    pq = psQKV.tile([P, CH], F32, tag="qkv")
                    for do in range(ND):
                        nc.tensor.matmul(
                            pq,
                            wq_t[:, do, :],
                            xTc[:, do, :],
                            start=(do == 0),
                            stop=(do == ND - 1),
                        )
                    nc.vector.tensor_copy(QTc[:, eb, :], pq)
                    pk = psQKV.tile([P, CH], F32, tag="qkv")
                    for do in range(ND):
                        nc.tensor.matmul(
                            pk,
                            wk_t[:, do, :],
                            xTc[:, do, :],
                            start=(do == 0),
                            stop=(do == ND - 1),
                        )
                    nc.vector.tensor_copy(KT[:, eb, c * CH : (c + 1) * CH], pk)

                for sb4 in range(QPC):
                    sb = c * QPC + sb4
                    xt_blk = xTc[:, :, sb4 * P : (sb4 + 1) * P]
                    pv0 = psQKV.tile([P, CH], F32, tag="qkv")
                    for do in range(ND):
                        nc.tensor.matmul(
                            pv0,
                            xt_blk[:, do, :],
                            WV[:, do, 0:CH],
                            start=(do == 0),
                            stop=(do == ND - 1),
                        )
                    nc.scalar.copy(V[:, sb, 0:CH], pv0)
                    pv1 = psQKV.tile([P, CH], F32, tag="qkv")
                    for do in range(ND):
                        nc.tensor.matmul(
                            pv1[:, 0 : D - CH],
                            xt_blk[:, do, :],
                            WV[:, do, CH:D],
                            start=(do == 0),
                            stop=(do == ND - 1),
                        )
                    nc.scalar.copy(V[:, sb, CH:D], pv1[:, 0 : D - CH])

                # ---- scores^T + exp (+ causal mask on partial blocks)
                # For diagonal blocks (kb = 4c+i, i>0) only q-cols >= i*128
                # are causally live and AV never reads the dead columns, so
                # narrow the matmul/exp/mask to the live width (min 256 to
                # stay on the f32r full-rate path).
                nkb = QPC * (c + 1)
                for kb in range(nkb):
                    i = kb - QPC * c
                    q0 = max(i, 0) * P
                    if CH - q0 < 256:
                        q0 = CH - 256
                    W = CH - q0
                    ps_s = psA.tile([P, CH], F32, tag="a")
                    for eo in range(ND):
                        nc.tensor.matmul(
                            ps_s[:, 0:W],
                            KT[:, eo, kb * P : (kb + 1) * P],
                            QTc[:, eo, q0:CH],
                            start=(eo == 0),
                            stop=(eo == ND - 1),
                        )
                    ptw = {13: 384, 14: 256, 15: 256}.get(kb, CH)
                    base = CH - ptw
                    pt = ptp.tile([P, ptw], MDT, tag=f"pt{kb}")
                    nc.scalar.activation(
                        pt[:, q0 - base : CH - base],
                        ps_s[:, 0:W],
                        mybir.ActivationFunctionType.Exp,
                        scale=SCALE,
                    )
                    pt_tiles[kb] = (pt, base)
                    if kb >= QPC * c:
                        off = c * CH - kb * P + 384
                        nc.vector.tensor_mul(
                            pt[:, q0 - base : CH - base],
                            pt[:, q0 - base : CH - base],
                            smask[:, off + q0 - 256 : off + CH - 256],
                        )

                # ---- attn @ [V | 1], normalize, store
                for qs in range(QPC):
                    qb = c * QPC + qs
                    po0 = psO.tile([P, CH], F32, tag="o")
                    po1 = psO.tile([P, CH], F32, tag="o")
                    for kb in range(qb + 1):
                        ptk, pbase = pt_tiles[kb]
                        lhs = ptk[:, qs * P - pbase : (qs + 1) * P - pbase]
                        nc.tensor.matmul(
                            po0,
                            lhs,
                            V[:, kb, 0:CH],
                            start=(kb == 0),
                            stop=(kb == qb),
                        )
                        nc.tensor.matmul(
                            po1[:, 0 : D + 2 - CH],
                            lhs,
                            V[:, kb, CH : D + 2],
                            start=(kb == 0),
                            stop=(kb == qb),
                        )
                    recip = rcp.tile([P, 1], F32, tag="rc")
                    nc.vector.reciprocal(recip, po1[:, D - CH : D - CH + 1])
                    o_sb = outp.tile([P, D], F32, tag="o")
                    nc.vector.tensor_scalar_mul(o_sb[:, 0:CH], po0, recip)
                    nc.vector.tensor_scalar_mul(
                        o_sb[:, CH:D], po1[:, 0 : D - CH], recip
                    )
                    nc.scalar.dma_start(out_d[qb * P : (qb + 1) * P, :], o_sb)

    nc.finalize()
    return nc


_NC_CACHE = None


def _get_nc():
    global _NC_CACHE
    if _NC_CACHE is None:
        _NC_CACHE = _build_nc()
    return _NC_CACHE


def run(inputs, trace=False):
    x = np.asarray(inputs["x"], dtype=np.float32)
    wq = np.asarray(inputs["wq"], dtype=np.float32)
    wk = np.asarray(inputs["wk"], dtype=np.float32)
    wv = np.asarray(inputs["wv"], dtype=np.float32)
    nc = _get_nc()
    in_maps = [
        {"xb": np.ascontiguousarray(x[b]), "wq": wq, "wk": wk, "wv": wv}
        for b in range(B)
    ]
    res = run_bass_kernel_spmd(nc, in_maps, core_ids=list(range(B)), trace=trace)
    out = np.stack([r["out"] for r in res.results]).astype(np.float32)
    return out, res


def kernel(x, wq, wk, wv):
    out, _ = run({"x": x, "wq": wq, "wk": wk, "wv": wv}, trace=False)
    return out

